# revision 15
# baseline (speedup 1.0000x reference)
# Trainium2 Bass kernel for nn_NetSparse1 (topk_masking).
#
# Computes: log_softmax( relu(x @ (w1*m1).T) @ (w2*m2).T ) where m1/m2 are
# top-50%-|score| masks (GetSubnetEP semantics, stable-sort tie handling).
#
# Strategy (data-parallel over 8 NeuronCores, batch dim sharded):
#   host: transpose/cast inputs (xT/w1T/scores bf16), compute the exact top-k
#         threshold t per layer (k-th order statistic of |scores|). The device
#         mask is (|bf16(s)| >= bf16(t)) which by rounding monotonicity keeps
#         a superset of the reference's kept set; the few extras (bf16
#         rounding band + stable-sort tie drops) are zeroed directly in the
#         bf16 weight copies on the host, making the masked weights exact.
#   device (per core, 2048 batch rows):
#     phase A: stream scores1T/w1T bf16, mask via one fused DVE op
#              (abs_max 0 -> is_ge t), w1m = mask * w1, resident in SBUF.
#     main:    for each 512-batch block: 64x hidden chunks of
#              psum[128h,512b] += w1m_chunk.T @ xT_chunk (7 K-chunks of 128),
#              relu->bf16 (ACT), then logitsT[10,512] += w2m_chunk.T @ h.
#              The logits matmul is software-pipelined one chunk behind so
#              the PE never stalls on the relu; each block's log_softmax
#              epilogue is emitted a few chunks into the next block to keep
#              the PE dense (HAM stays warm).
#     epilog:  PE-transpose logitsT to [128b,10], log_softmax along the free
#              dim (max-shifted, like jax), DMA out.
# No collectives needed; host concatenates the 8 per-core outputs.

import numpy as np
import ml_dtypes

import concourse.bass as bass
import concourse.tile as tile
from concourse import bacc, mybir
from concourse.bass_utils import run_bass_kernel_spmd
from concourse.masks import make_identity

N_CORES = 8
B = 16384
BC = B // N_CORES      # 2048 batch rows per core
IN_DIM = 784
HIDDEN = 8192
OUT_DIM = 10
SPARSITY = 0.5

P = 128
KC = 7                 # ceil(784/128) contraction chunks
K_LAST = IN_DIM - 6 * P  # 16
HC = HIDDEN // P       # 64 hidden chunks
BB = 512               # batch block (PSUM free dim)
NBB = BC // BB         # 4
CB = 1024              # phase-A column piece over hidden
NCB = HIDDEN // CB     # 8
HC_PER_CB = CB // P    # 8

F32 = mybir.dt.float32
BF16 = mybir.dt.bfloat16

_BF16 = ml_dtypes.bfloat16


def _build_nc():
    nc = bacc.Bacc("TRN2")

    xT = nc.dram_tensor("xT", (IN_DIM, BC), BF16, kind="ExternalInput")
    w1T = nc.dram_tensor("w1T", (IN_DIM, HIDDEN), BF16, kind="ExternalInput")
    s1T = nc.dram_tensor("s1T", (IN_DIM, HIDDEN), BF16, kind="ExternalInput")
    w2T = nc.dram_tensor("w2T", (HIDDEN, OUT_DIM), BF16, kind="ExternalInput")
    s2T = nc.dram_tensor("s2T", (HIDDEN, OUT_DIM), BF16, kind="ExternalInput")
    # [t1, t2, -t1, -t2]
    ths = nc.dram_tensor("ths", (1, 4), F32, kind="ExternalInput")
    out = nc.dram_tensor("out", (BC, OUT_DIM), F32, kind="ExternalOutput")

    with tile.TileContext(nc) as tc:
        with (
            tc.tile_pool(name="singles", bufs=1) as singles,
            tc.tile_pool(name="wres", bufs=1) as wres,
            tc.tile_pool(name="stream", bufs=3) as stream,
            tc.tile_pool(name="w2p", bufs=1) as w2p,
            tc.tile_pool(name="hpool", bufs=4) as hpool,
            tc.tile_pool(name="opool", bufs=4) as opool,
            tc.tile_pool(name="tailp", bufs=1) as tailp,
            tc.tile_pool(name="psh", bufs=2, space=bass.MemorySpace.PSUM) as psh,
            tc.tile_pool(name="psl", bufs=1, space=bass.MemorySpace.PSUM) as psl,
            tc.tile_pool(name="pst", bufs=2, space=bass.MemorySpace.PSUM) as pst,
        ):
            # thresholds broadcast across partitions: [128, 4]
            t_bc = singles.tile([P, 4], F32, tag="t_bc")
            nc.sync.dma_start(t_bc, bass.AP(ths, 0, [[0, P], [1, 4]]))

            # zero bias for activations
            zb = singles.tile([P, 1], F32, tag="zb")
            nc.vector.memset(zb, 0.0)

            # identity for PE transpose
            ident = singles.tile([P, P], F32, tag="ident")
            make_identity(nc, ident[:])

            # PE warmup: dependency-free matmul chain so the HAM clock-gate
            # is at K=8/8 by the time the first real matmul's inputs arrive
            warm = psh.tile([P, 64], F32, tag="ph")
            for i in range(300):
                nc.tensor.matmul(warm, ident, ident[:, :64],
                                 start=(i == 0), stop=(i == 299))

            def phase_a_piece(cb, kc, w1m):
                pk = P if kc < KC - 1 else K_LAST
                dst = wres.tile([P, CB], BF16, tag=f"w1m_{kc}_{cb}")
                if pk < P:
                    nc.vector.memset(dst, 0.0)
                sc = stream.tile([P, CB], BF16, tag="sc")
                nc.sync.dma_start(
                    sc[:pk], s1T[kc * P : kc * P + pk, cb * CB : (cb + 1) * CB])
                # mask = (s >= t) + (s <= -t), all on DVE (keeps ACT free
                # for relu: no activation-table swaps mid-kernel)
                ge = stream.tile([P, CB], BF16, tag="ge")
                nc.vector.tensor_scalar(out=ge[:pk], in0=sc[:pk],
                                        scalar1=t_bc[:pk, 0:1], scalar2=None,
                                        op0=mybir.AluOpType.is_ge)
                gl = stream.tile([P, CB], BF16, tag="gl")
                nc.vector.tensor_scalar(out=gl[:pk], in0=sc[:pk],
                                        scalar1=t_bc[:pk, 2:3], scalar2=None,
                                        op0=mybir.AluOpType.is_le)
                nc.vector.tensor_add(out=ge[:pk], in0=ge[:pk], in1=gl[:pk])
                wt = stream.tile([P, CB], BF16, tag="wt")
                nc.gpsimd.dma_start(
                    wt[:pk], w1T[kc * P : kc * P + pk, cb * CB : (cb + 1) * CB])
                nc.vector.tensor_mul(dst[:pk], ge[:pk], wt[:pk])
                w1m[kc][cb] = dst

            w1m = [[None] * NCB for _ in range(KC)]
            # first column piece + xT up front so the PE can start early
            for kc in range(KC):
                phase_a_piece(0, kc, w1m)

            xs = [[None] * NBB for _ in range(KC)]
            for bbi in range(NBB):
                for kc in range(KC):
                    pk = P if kc < KC - 1 else K_LAST
                    xt = wres.tile([P, BB], BF16, tag=f"x_{kc}_{bbi}")
                    if pk < P:
                        nc.vector.memset(xt, 0.0)
                    nc.sync.dma_start(
                        xt[:pk, :], xT[kc * P : kc * P + pk,
                                       bbi * BB : (bbi + 1) * BB])
                    xs[kc][bbi] = xt
            for cb in range(1, NCB):
                for kc in range(KC):
                    phase_a_piece(cb, kc, w1m)

            # masked w2 (resident): [128, 64, 10] bf16
            w2m = singles.tile([P, HC, OUT_DIM], BF16, tag="w2m")
            s2_t = w2p.tile([P, HC, OUT_DIM], BF16, tag="s2_t")
            w2_t = w2p.tile([P, HC, OUT_DIM], BF16, tag="w2_t")
            ge2 = w2p.tile([P, HC, OUT_DIM], BF16, tag="ge2")
            nc.sync.dma_start(s2_t, s2T[:].rearrange("(c p) o -> p c o", p=P))
            nc.gpsimd.dma_start(w2_t, w2T[:].rearrange("(c p) o -> p c o", p=P))
            gl2 = w2p.tile([P, HC, OUT_DIM], BF16, tag="gl2")
            nc.vector.tensor_scalar(out=ge2, in0=s2_t,
                                    scalar1=t_bc[:, 1:2], scalar2=None,
                                    op0=mybir.AluOpType.is_ge)
            nc.vector.tensor_scalar(out=gl2, in0=s2_t,
                                    scalar1=t_bc[:, 3:4], scalar2=None,
                                    op0=mybir.AluOpType.is_le)
            nc.vector.tensor_add(out=ge2, in0=ge2, in1=gl2)
            nc.vector.tensor_mul(w2m, ge2, w2_t)

            # main compute: hc-outer / bb-inner so one phase-A column piece
            # feeds ~55us of PE work (8 hc x 4 bb); the logits matmul for
            # (hc,bb) is deferred one step so the PE never waits on the relu
            lgs = [psl.tile([OUT_DIM, BB], F32, tag=f"lg_{b}", name=f"lg_{b}")
                   for b in range(NBB)]
            pend = None  # (ht, hc, bb) whose logits matmul is deferred
            for hc in range(HC):
                for bb in range(NBB):
                    ph = psh.tile([P, BB], F32, tag="ph")
                    for kc in range(KC):
                        nc.tensor.matmul(
                            ph,
                            w1m[kc][hc // HC_PER_CB][:, (hc % HC_PER_CB) * P :
                                                     (hc % HC_PER_CB) * P + P],
                            xs[kc][bb],
                            start=(kc == 0),
                            stop=(kc == KC - 1),
                        )
                    ht = hpool.tile([P, BB], BF16, tag="ht")
                    nc.scalar.activation(out=ht, in_=ph,
                                         func=mybir.ActivationFunctionType.Relu,
                                         bias=zb)
                    if pend is not None:
                        p_ht, p_hc, p_bb = pend
                        nc.tensor.matmul(lgs[p_bb], w2m[:, p_hc, :], p_ht,
                                         start=(p_hc == 0), stop=(p_hc == HC - 1))
                    pend = (ht, hc, bb)
            p_ht, p_hc, p_bb = pend
            nc.tensor.matmul(lgs[p_bb], w2m[:, p_hc, :], p_ht,
                             start=False, stop=True)

            # tail: log_softmax for all 16 [128,10] tiles, phased to avoid
            # ACT table swaps (all Exp together, one Ln over [128,16])
            lg_sbs = []
            for bb in range(NBB):
                lg_sb = tailp.tile([OUT_DIM, BB], F32, tag=f"lg_sb_{bb}")
                nc.vector.tensor_copy(lg_sb, lgs[bb])
                lg_sbs.append(lg_sb)
            NT = NBB * (BB // P)  # 16 tiles of [128, 10]
            xm_all = tailp.tile([P, NT, OUT_DIM], F32, tag="xm_all")
            e_all = tailp.tile([P, NT, OUT_DIM], F32, tag="e_all")
            s_all = tailp.tile([P, NT], F32, tag="s_all")
            ls_all = tailp.tile([P, NT], F32, tag="ls_all")
            ot_all = tailp.tile([P, NT, OUT_DIM], F32, tag="ot_all")
            for i in range(NT):
                bb, bs = divmod(i, BB // P)
                pt = pst.tile([P, OUT_DIM], F32, tag="pt")
                nc.tensor.transpose(pt, lg_sbs[bb][:, bs * P : (bs + 1) * P],
                                    ident[:OUT_DIM, :OUT_DIM])
                mx = opool.tile([P, 1], F32, tag="mx")
                nc.vector.reduce_max(out=mx, in_=pt, axis=mybir.AxisListType.X)
                nc.vector.tensor_scalar(out=xm_all[:, i, :], in0=pt,
                                        scalar1=mx, scalar2=None,
                                        op0=mybir.AluOpType.subtract)
            for i in range(NT):
                nc.scalar.activation(out=e_all[:, i, :], in_=xm_all[:, i, :],
                                     func=mybir.ActivationFunctionType.Exp,
                                     bias=zb, accum_out=s_all[:, i : i + 1])
            nc.scalar.activation(out=ls_all, in_=s_all,
                                 func=mybir.ActivationFunctionType.Ln, bias=zb)
            for i in range(NT):
                nc.vector.tensor_scalar(out=ot_all[:, i, :],
                                        in0=xm_all[:, i, :],
                                        scalar1=ls_all[:, i : i + 1],
                                        scalar2=None,
                                        op0=mybir.AluOpType.subtract)
            nc.gpsimd.dma_start(out[:].rearrange("(i p) o -> p i o", p=P),
                                ot_all)

    nc.compile()
    return nc


_NC = None


def _get_nc():
    global _NC
    if _NC is None:
        _NC = _build_nc()
    return _NC


def _exact_mask_threshold(scores, wT_bf16):
    """GetSubnetEP mask, made exact for the device's bf16 compare.

    Reference keeps the top (n - j) entries of |scores| under stable-sort
    (value, flat-index) order, j = int((1-k)*n). The device keeps
    |bf16(s)| >= bf16(t) (t = j-th order statistic), a superset by rounding
    monotonicity; every extra entry is zeroed in wT_bf16 (transposed layout).
    Returns the f32 value of bf16(t) for the device compare.
    """
    s32 = np.asarray(scores, dtype=np.float32)
    a = np.abs(s32).ravel()
    n = a.size
    j = int((1.0 - SPARSITY) * n)
    t = np.partition(a, j)[j]
    lt = int((a < t).sum())
    ties = np.flatnonzero(a == t)  # ascending flat index == stable order
    mask_ref = a > t
    mask_ref[ties[j - lt :]] = True

    ab = np.abs(s32.astype(_BF16).astype(np.float32)).ravel()
    t_bf = np.float32(np.float32(t).astype(_BF16).astype(np.float32))
    mask_dev = ab >= t_bf
    assert not np.any(mask_ref & ~mask_dev), "device mask dropped a kept entry"
    extra = np.flatnonzero(mask_dev & ~mask_ref)
    ncols = scores.shape[1]
    wT_bf16[extra % ncols, extra // ncols] = 0
    assert int(mask_ref.sum()) == n - j
    return t_bf


def _prepare_inputs(x, w1, scores1, w2, scores2):
    x = np.asarray(x, dtype=np.float32)
    w1 = np.asarray(w1, dtype=np.float32)
    w2 = np.asarray(w2, dtype=np.float32)

    w1T = np.ascontiguousarray(w1.T).astype(_BF16)   # [784, 8192]
    w2T = np.ascontiguousarray(w2.T).astype(_BF16)   # [8192, 10]
    t1 = _exact_mask_threshold(scores1, w1T)
    t2 = _exact_mask_threshold(scores2, w2T)

    s1T = np.ascontiguousarray(np.asarray(scores1, np.float32).T).astype(_BF16)
    s2T = np.ascontiguousarray(np.asarray(scores2, np.float32).T).astype(_BF16)
    xTb = np.ascontiguousarray(x.T).astype(_BF16)    # [784, 16384]
    ths = np.array([[t1, t2, -t1, -t2]], dtype=np.float32)

    common = {"w1T": w1T, "s1T": s1T, "w2T": w2T, "s2T": s2T, "ths": ths}
    in_maps = []
    for c in range(N_CORES):
        m = dict(common)
        m["xT"] = np.ascontiguousarray(xTb[:, c * BC : (c + 1) * BC])
        in_maps.append(m)
    return in_maps


def run(inputs, trace=False, **kwargs):
    """Run the kernel; returns (output ndarray, BassKernelResults)."""
    nc = _get_nc()
    in_maps = _prepare_inputs(**inputs)
    res = run_bass_kernel_spmd(nc, in_maps, core_ids=list(range(N_CORES)),
                               trace=trace, **kwargs)
    outp = np.concatenate([r["out"] for r in res.results], axis=0)
    return np.ascontiguousarray(outp.astype(np.float32)), res


def kernel(x, w1, scores1, w2, scores2):
    outp, _ = run(dict(x=x, w1=w1, scores1=scores1, w2=w2, scores2=scores2))
    return outp


# revision 17
# speedup vs baseline: 1.2086x; 1.2086x over previous
# Trainium2 Bass kernel for nn_NetSparse1 (topk_masking).
#
# Computes: log_softmax( relu(x @ (w1*m1).T) @ (w2*m2).T ) where m1/m2 are
# top-50%-|score| masks (GetSubnetEP semantics, stable-sort tie handling).
#
# Strategy (data-parallel over 8 NeuronCores, batch dim sharded):
#   host: transpose/cast inputs (xT/w1T/scores bf16), compute the exact top-k
#         threshold t per layer (k-th order statistic of |scores|). The device
#         mask is (|bf16(s)| >= bf16(t)) which by rounding monotonicity keeps
#         a superset of the reference's kept set; the few extras (bf16
#         rounding band + stable-sort tie drops) are zeroed directly in the
#         bf16 weight copies on the host, making the masked weights exact.
#   device (per core, 2048 batch rows):
#     phase A: stream scores1T/w1T bf16, mask via one fused DVE op
#              (abs_max 0 -> is_ge t), w1m = mask * w1, resident in SBUF.
#     main:    for each 512-batch block: 64x hidden chunks of
#              psum[128h,512b] += w1m_chunk.T @ xT_chunk (7 K-chunks of 128),
#              relu->bf16 (ACT), then logitsT[10,512] += w2m_chunk.T @ h.
#              The logits matmul is software-pipelined one chunk behind so
#              the PE never stalls on the relu; each block's log_softmax
#              epilogue is emitted a few chunks into the next block to keep
#              the PE dense (HAM stays warm).
#     epilog:  PE-transpose logitsT to [128b,10], log_softmax along the free
#              dim (max-shifted, like jax), DMA out.
# No collectives needed; host concatenates the 8 per-core outputs.

import numpy as np
import ml_dtypes

import concourse.bass as bass
import concourse.tile as tile
from concourse import bacc, mybir
from concourse.bass_utils import run_bass_kernel_spmd
from concourse.masks import make_identity

N_CORES = 8
B = 16384
BC = B // N_CORES      # 2048 batch rows per core
IN_DIM = 784
HIDDEN = 8192
OUT_DIM = 10
SPARSITY = 0.5

P = 128
KC = 7                 # ceil(784/128) contraction chunks
K_LAST = IN_DIM - 6 * P  # 16
HC = HIDDEN // P       # 64 hidden chunks
BB = 512               # batch block (PSUM free dim)
NBB = BC // BB         # 4
CB = 1024              # phase-A column piece over hidden
NCB = HIDDEN // CB     # 8
HC_PER_CB = CB // P    # 8

F32 = mybir.dt.float32
BF16 = mybir.dt.bfloat16

_BF16 = ml_dtypes.bfloat16


def _build_nc():
    nc = bacc.Bacc("TRN2")

    xT = nc.dram_tensor("xT", (IN_DIM, BC), BF16, kind="ExternalInput")
    w1T = nc.dram_tensor("w1T", (IN_DIM, HIDDEN), BF16, kind="ExternalInput")
    s1T = nc.dram_tensor("s1T", (IN_DIM, HIDDEN), BF16, kind="ExternalInput")
    w2T = nc.dram_tensor("w2T", (HIDDEN, OUT_DIM), BF16, kind="ExternalInput")
    s2T = nc.dram_tensor("s2T", (HIDDEN, OUT_DIM), BF16, kind="ExternalInput")
    # [t1, t2, -t1, -t2]
    ths = nc.dram_tensor("ths", (1, 4), F32, kind="ExternalInput")
    out = nc.dram_tensor("out", (BC, OUT_DIM), F32, kind="ExternalOutput")

    with tile.TileContext(nc) as tc:
        with (
            tc.tile_pool(name="singles", bufs=1) as singles,
            tc.tile_pool(name="wres", bufs=1) as wres,
            tc.tile_pool(name="stream", bufs=3) as stream,
            tc.tile_pool(name="w2p", bufs=1) as w2p,
            tc.tile_pool(name="hpool", bufs=4) as hpool,
            tc.tile_pool(name="opool", bufs=4) as opool,
            tc.tile_pool(name="tailp", bufs=1) as tailp,
            tc.tile_pool(name="psh", bufs=2, space=bass.MemorySpace.PSUM) as psh,
            tc.tile_pool(name="psl", bufs=1, space=bass.MemorySpace.PSUM) as psl,
            tc.tile_pool(name="pst", bufs=2, space=bass.MemorySpace.PSUM) as pst,
        ):
            # thresholds broadcast across partitions: [128, 4]
            t_bc = singles.tile([P, 4], F32, tag="t_bc")
            nc.sync.dma_start(t_bc, bass.AP(ths, 0, [[0, P], [1, 4]]))

            # zero bias for activations
            zb = singles.tile([P, 1], F32, tag="zb")
            nc.vector.memset(zb, 0.0)

            # identity for PE transpose
            ident = singles.tile([P, P], F32, tag="ident")
            make_identity(nc, ident[:])

            # PE warmup: dependency-free bf16 matmul chain so the HAM
            # clock-gate is at K=8/8 when the first real matmul's inputs land
            wz = singles.tile([P, P], BF16, tag="wz")
            nc.vector.memset(wz, 0.0)
            warm = psh.tile([P, 64], F32, tag="ph")
            for i in range(150):
                nc.tensor.matmul(warm, wz, wz[:, :64],
                                 start=(i == 0), stop=(i == 149))

            def phase_a_piece(cb, kc, w1m):
                pk = P if kc < KC - 1 else K_LAST
                dst = wres.tile([P, CB], BF16, tag=f"w1m_{kc}_{cb}")
                if pk < P:
                    nc.vector.memset(dst, 0.0)
                sc = stream.tile([P, CB], BF16, tag="sc")
                nc.sync.dma_start(
                    sc[:pk], s1T[kc * P : kc * P + pk, cb * CB : (cb + 1) * CB])
                # mask = (s >= t) + (s <= -t), all on DVE (keeps ACT free
                # for relu: no activation-table swaps mid-kernel)
                ge = stream.tile([P, CB], BF16, tag="ge")
                nc.vector.tensor_scalar(out=ge[:pk], in0=sc[:pk],
                                        scalar1=t_bc[:pk, 0:1], scalar2=None,
                                        op0=mybir.AluOpType.is_ge)
                gl = stream.tile([P, CB], BF16, tag="gl")
                nc.vector.tensor_scalar(out=gl[:pk], in0=sc[:pk],
                                        scalar1=t_bc[:pk, 2:3], scalar2=None,
                                        op0=mybir.AluOpType.is_le)
                nc.vector.tensor_add(out=ge[:pk], in0=ge[:pk], in1=gl[:pk])
                wt = stream.tile([P, CB], BF16, tag="wt")
                nc.gpsimd.dma_start(
                    wt[:pk], w1T[kc * P : kc * P + pk, cb * CB : (cb + 1) * CB])
                nc.vector.tensor_mul(dst[:pk], ge[:pk], wt[:pk])
                w1m[kc][cb] = dst

            # masked w2 (resident): [128, 64, 10] bf16
            w2m = singles.tile([P, HC, OUT_DIM], BF16, tag="w2m")
            s2_t = w2p.tile([P, HC, OUT_DIM], BF16, tag="s2_t")
            w2_t = w2p.tile([P, HC, OUT_DIM], BF16, tag="w2_t")
            ge2 = w2p.tile([P, HC, OUT_DIM], BF16, tag="ge2")
            nc.sync.dma_start(s2_t, s2T[:].rearrange("(c p) o -> p c o", p=P))
            nc.gpsimd.dma_start(w2_t, w2T[:].rearrange("(c p) o -> p c o", p=P))
            gl2 = w2p.tile([P, HC, OUT_DIM], BF16, tag="gl2")
            nc.vector.tensor_scalar(out=ge2, in0=s2_t,
                                    scalar1=t_bc[:, 1:2], scalar2=None,
                                    op0=mybir.AluOpType.is_ge)
            nc.vector.tensor_scalar(out=gl2, in0=s2_t,
                                    scalar1=t_bc[:, 3:4], scalar2=None,
                                    op0=mybir.AluOpType.is_le)
            nc.vector.tensor_add(out=ge2, in0=ge2, in1=gl2)
            nc.vector.tensor_mul(w2m, ge2, w2_t)

            w1m = [[None] * NCB for _ in range(KC)]
            # first column piece + xT up front so the PE can start early
            for kc in range(KC):
                phase_a_piece(0, kc, w1m)

            xs = [[None] * NBB for _ in range(KC)]
            for bbi in range(NBB):
                for kc in range(KC):
                    pk = P if kc < KC - 1 else K_LAST
                    xt = wres.tile([P, BB], BF16, tag=f"x_{kc}_{bbi}")
                    if pk < P:
                        nc.vector.memset(xt, 0.0)
                    nc.sync.dma_start(
                        xt[:pk, :], xT[kc * P : kc * P + pk,
                                       bbi * BB : (bbi + 1) * BB])
                    xs[kc][bbi] = xt
            for cb in range(1, NCB):
                for kc in range(KC):
                    phase_a_piece(cb, kc, w1m)

            # main compute: hc-outer / bb-inner so one phase-A column piece
            # feeds ~55us of PE work (8 hc x 4 bb); the logits matmul for
            # (hc,bb) is deferred one step so the PE never waits on the relu
            lgs = [psl.tile([OUT_DIM, BB], F32, tag=f"lg_{b}", name=f"lg_{b}")
                   for b in range(NBB)]
            pend = None  # (ht, hc, bb) whose logits matmul is deferred
            for hc in range(HC):
                for bb in range(NBB):
                    ph = psh.tile([P, BB], F32, tag="ph")
                    for kc in range(KC):
                        nc.tensor.matmul(
                            ph,
                            w1m[kc][hc // HC_PER_CB][:, (hc % HC_PER_CB) * P :
                                                     (hc % HC_PER_CB) * P + P],
                            xs[kc][bb],
                            start=(kc == 0),
                            stop=(kc == KC - 1),
                        )
                    ht = hpool.tile([P, BB], BF16, tag="ht")
                    nc.scalar.activation(out=ht, in_=ph,
                                         func=mybir.ActivationFunctionType.Relu,
                                         bias=zb)
                    if pend is not None:
                        p_ht, p_hc, p_bb = pend
                        nc.tensor.matmul(lgs[p_bb], w2m[:, p_hc, :], p_ht,
                                         start=(p_hc == 0), stop=(p_hc == HC - 1))
                    pend = (ht, hc, bb)
            p_ht, p_hc, p_bb = pend
            nc.tensor.matmul(lgs[p_bb], w2m[:, p_hc, :], p_ht,
                             start=False, stop=True)

            # tail: log_softmax for all 16 [128,10] tiles, phased to avoid
            # ACT table swaps (all Exp together, one Ln over [128,16])
            lg_sbs = []
            for bb in range(NBB):
                lg_sb = tailp.tile([OUT_DIM, BB], F32, tag=f"lg_sb_{bb}")
                nc.vector.tensor_copy(lg_sb, lgs[bb])
                lg_sbs.append(lg_sb)
            NT = NBB * (BB // P)  # 16 tiles of [128, 10]
            xm_all = tailp.tile([P, NT, OUT_DIM], F32, tag="xm_all")
            e_all = tailp.tile([P, NT, OUT_DIM], F32, tag="e_all")
            s_all = tailp.tile([P, NT], F32, tag="s_all")
            ls_all = tailp.tile([P, NT], F32, tag="ls_all")
            ot_all = tailp.tile([P, NT, OUT_DIM], F32, tag="ot_all")
            for i in range(NT):
                bb, bs = divmod(i, BB // P)
                pt = pst.tile([P, OUT_DIM], F32, tag="pt")
                nc.tensor.transpose(pt, lg_sbs[bb][:, bs * P : (bs + 1) * P],
                                    ident[:OUT_DIM, :OUT_DIM])
                mx = opool.tile([P, 1], F32, tag="mx")
                nc.vector.reduce_max(out=mx, in_=pt, axis=mybir.AxisListType.X)
                nc.vector.tensor_scalar(out=xm_all[:, i, :], in0=pt,
                                        scalar1=mx, scalar2=None,
                                        op0=mybir.AluOpType.subtract)
            for i in range(NT):
                nc.scalar.activation(out=e_all[:, i, :], in_=xm_all[:, i, :],
                                     func=mybir.ActivationFunctionType.Exp,
                                     bias=zb, accum_out=s_all[:, i : i + 1])
            nc.scalar.activation(out=ls_all, in_=s_all,
                                 func=mybir.ActivationFunctionType.Ln, bias=zb)
            for i in range(NT):
                nc.vector.tensor_scalar(out=ot_all[:, i, :],
                                        in0=xm_all[:, i, :],
                                        scalar1=ls_all[:, i : i + 1],
                                        scalar2=None,
                                        op0=mybir.AluOpType.subtract)
            nc.gpsimd.dma_start(out[:].rearrange("(i p) o -> p i o", p=P),
                                ot_all)

    nc.compile()
    return nc


_NC = None


def _get_nc():
    global _NC
    if _NC is None:
        _NC = _build_nc()
    return _NC


def _exact_mask_threshold(scores, wT_bf16):
    """GetSubnetEP mask, made exact for the device's bf16 compare.

    Reference keeps the top (n - j) entries of |scores| under stable-sort
    (value, flat-index) order, j = int((1-k)*n). The device keeps
    |bf16(s)| >= bf16(t) (t = j-th order statistic), a superset by rounding
    monotonicity; every extra entry is zeroed in wT_bf16 (transposed layout).
    Returns the f32 value of bf16(t) for the device compare.
    """
    s32 = np.asarray(scores, dtype=np.float32)
    a = np.abs(s32).ravel()
    n = a.size
    j = int((1.0 - SPARSITY) * n)
    t = np.partition(a, j)[j]
    lt = int((a < t).sum())
    ties = np.flatnonzero(a == t)  # ascending flat index == stable order
    mask_ref = a > t
    mask_ref[ties[j - lt :]] = True

    ab = np.abs(s32.astype(_BF16).astype(np.float32)).ravel()
    t_bf = np.float32(np.float32(t).astype(_BF16).astype(np.float32))
    mask_dev = ab >= t_bf
    assert not np.any(mask_ref & ~mask_dev), "device mask dropped a kept entry"
    extra = np.flatnonzero(mask_dev & ~mask_ref)
    ncols = scores.shape[1]
    wT_bf16[extra % ncols, extra // ncols] = 0
    assert int(mask_ref.sum()) == n - j
    return t_bf


def _prepare_inputs(x, w1, scores1, w2, scores2):
    x = np.asarray(x, dtype=np.float32)
    w1 = np.asarray(w1, dtype=np.float32)
    w2 = np.asarray(w2, dtype=np.float32)

    w1T = np.ascontiguousarray(w1.T).astype(_BF16)   # [784, 8192]
    w2T = np.ascontiguousarray(w2.T).astype(_BF16)   # [8192, 10]
    t1 = _exact_mask_threshold(scores1, w1T)
    t2 = _exact_mask_threshold(scores2, w2T)

    s1T = np.ascontiguousarray(np.asarray(scores1, np.float32).T).astype(_BF16)
    s2T = np.ascontiguousarray(np.asarray(scores2, np.float32).T).astype(_BF16)
    xTb = np.ascontiguousarray(x.T).astype(_BF16)    # [784, 16384]
    ths = np.array([[t1, t2, -t1, -t2]], dtype=np.float32)

    common = {"w1T": w1T, "s1T": s1T, "w2T": w2T, "s2T": s2T, "ths": ths}
    in_maps = []
    for c in range(N_CORES):
        m = dict(common)
        m["xT"] = np.ascontiguousarray(xTb[:, c * BC : (c + 1) * BC])
        in_maps.append(m)
    return in_maps


def run(inputs, trace=False, **kwargs):
    """Run the kernel; returns (output ndarray, BassKernelResults)."""
    nc = _get_nc()
    in_maps = _prepare_inputs(**inputs)
    res = run_bass_kernel_spmd(nc, in_maps, core_ids=list(range(N_CORES)),
                               trace=trace, **kwargs)
    outp = np.concatenate([r["out"] for r in res.results], axis=0)
    return np.ascontiguousarray(outp.astype(np.float32)), res


def kernel(x, w1, scores1, w2, scores2):
    outp, _ = run(dict(x=x, w1=w1, scores1=scores1, w2=w2, scores2=scores2))
    return outp
